# revision 1
# baseline (speedup 1.0000x reference)
"""KGATConv GNN message-passing kernel for 8 Trainium2 NeuronCores.

Strategy (src-node sharding + on-device ReduceScatter; wire-optimized):
  - Core k owns nodes [k*12500, (k+1)*12500) and holds ONLY its nfeat shard
    (fp16 on the wire, padded to 12544 rows) -- no replication.  Each edge is
    routed to the core owning its src node, so the per-chunk indirect gather
    reads the local shard.
  - Host buckets each core's edges by global dst window (784 windows of 128
    padded dst rows), padding each window's edge run to whole 128-edge chunks
    (chunk counts shared across cores so all 8 run one SPMD program).  Edge
    tables ship as int16 src / uint8 window-offset / fp16 weight and are
    widened on device.
  - Device, per window: per chunk, indirect-DMA gather of 128 local nfeat
    rows (one offset per partition); DVE builds A[p,j] = w_p * (dstoff_p==j);
    PE matmul-accumulates partial h_nb = A^T @ g in PSUM (fp16 in, f32 acc);
    PSUM is copied out to a [100352,128] f32 DRAM partial buffer.
  - One ReduceScatter(add) over all 8 cores turns per-core partials into
    each core's owned h_neighbor rows [12544,128].
  - Finalize per own-window: X = nfeat_own * h_nb, X^T via PE transpose,
    out = X @ W^T on PE (f32), LeakyReLU on ACT, fp16 out on the wire.
"""

import sys

sys.path.insert(0, "/opt/trn_rl_repo")

from concurrent.futures import ThreadPoolExecutor
from contextlib import ExitStack

import numpy as np
import jax

# Persistent compilation cache: without it every kernel() call re-enters
# neuronx_cc_hook (walrus birverifier subprocess + DVE table gen, ~1.3s)
# because each run_bass_kernel_spmd call builds a fresh jax.jit closure.
for _k, _v in (
    ("jax_compilation_cache_dir", "/tmp/jax_pcc"),
    ("jax_persistent_cache_min_compile_time_secs", 0),
    ("jax_persistent_cache_min_entry_size_bytes", 0),
):
    try:
        jax.config.update(_k, _v)
    except Exception:
        pass

import concourse.bass as bass
import concourse.mybir as mybir
import concourse.tile as tile
from concourse.bass_utils import run_bass_kernel_spmd

N_CORES = 8
D = 128
WIN = 128
NPC = 12500  # nodes owned per core
PAD = 12544  # NPC rounded up to a whole number of 128-row windows
NW_OWN = PAD // WIN  # 98 windows of owned nodes per core
NW = N_CORES * NW_OWN  # 784 global dst windows

_nc_cache = {}
_pool = ThreadPoolExecutor(max_workers=N_CORES)


def _split_excess_waits(nc, maxw=1):
    # This walrus build rejects instructions carrying more than one sync
    # wait.  Move extras onto the immediately preceding instruction of the
    # same engine+queue when it has a free wait slot (engine queues are
    # in-order, so hoisting a monotonic-semaphore wait one slot earlier is
    # equivalent to the NoOp the fallback inserts); otherwise insert NoOps.
    def qkey(i):
        return (i.engine, getattr(i, "queue", None))

    for f in nc.m.functions:
        for bb in f.blocks:
            out = []
            for inst in bb.instructions:
                si = inst.sync_info
                waits = list(si.on_wait) if si and si.on_wait else []
                if len(waits) > maxw:
                    extra, keep = waits[:-maxw], waits[-maxw:]
                    # hoist onto the directly preceding run of same-queue
                    # instructions with free wait slots
                    k = len(out) - 1
                    while extra and k >= 0 and qkey(out[k]) == qkey(inst):
                        psi = out[k].sync_info
                        pw = list(psi.on_wait) if psi and psi.on_wait else []
                        room = maxw - len(pw)
                        if room <= 0:
                            break
                        take, extra = extra[-room:], extra[:-room]
                        if psi is None:
                            out[k].sync_info = type(si)(
                                on_wait=list(take), on_update=[]
                            )
                        else:
                            psi.on_wait = pw + list(take)
                        k -= 1
                    for i in range(0, len(extra), maxw):
                        nop = mybir.InstNoOp(
                            name=nc.get_next_instruction_name(), ins=[], outs=[]
                        )
                        nop.engine = inst.engine
                        nop.sync_info = type(si)(
                            on_wait=extra[i : i + maxw], on_update=[]
                        )
                        nc.register_instruction(nop, overwrite=True)
                        out.append(nop)
                    si.on_wait = keep
                out.append(inst)
            bb.instructions[:] = out


def _build_nc(ct, c_list):
    f32 = mybir.dt.float32
    f16 = mybir.dt.float16
    i32 = mybir.dt.int32
    nc = bass.Bass(num_devices=N_CORES)
    nfeat_d = nc.declare_dram_parameter("nfeat", [PAD, D], mybir.dt.int8, isOutput=False)
    scale_d = nc.declare_dram_parameter("scl", [128, NW_OWN], f32, isOutput=False)
    src_d = nc.declare_dram_parameter("src", [128, ct], mybir.dt.int16, isOutput=False)
    off_d = nc.declare_dram_parameter("offs", [128, ct], mybir.dt.uint8, isOutput=False)
    w_d = nc.declare_dram_parameter("wf", [128, ct], f16, isOutput=False)
    wt_d = nc.declare_dram_parameter("wt", [D, D], f32, isOutput=False)
    out_d = nc.declare_dram_parameter("out", [PAD, D], mybir.dt.int8, isOutput=True)
    outs_d = nc.declare_dram_parameter("outs", [128, NW_OWN], f32, isOutput=True)

    with tile.TileContext(nc) as tc, ExitStack() as ctx:
        const = ctx.enter_context(tc.tile_pool(name="const", bufs=1))
        gp = ctx.enter_context(tc.tile_pool(name="gp", bufs=10))
        ap = ctx.enter_context(tc.tile_pool(name="ap", bufs=4))
        wk = ctx.enter_context(tc.tile_pool(name="wk", bufs=3))
        ps = ctx.enter_context(tc.tile_pool(name="ps", bufs=2, space="PSUM"))
        dram = ctx.enter_context(tc.tile_pool(name="dram", bufs=1, space="DRAM"))

        src16 = const.tile([128, ct], mybir.dt.int16)
        nc.sync.dma_start(out=src16[:], in_=src_d[:])
        scale_sb = const.tile([128, NW_OWN], f32)
        nc.sync.dma_start(out=scale_sb[:], in_=scale_d[:])
        off8 = const.tile([128, ct], mybir.dt.uint8)
        nc.sync.dma_start(out=off8[:], in_=off_d[:])
        w16 = const.tile([128, ct], f16)
        nc.sync.dma_start(out=w16[:], in_=w_d[:])
        wt_sb = const.tile([D, D], f32)
        nc.sync.dma_start(out=wt_sb[:], in_=wt_d[:])

        # widen the wire-compressed edge tables once
        src_sb = const.tile([128, ct], i32)
        nc.vector.tensor_scalar(src_sb[:], src16[:], 0, None, mybir.AluOpType.add)
        off_sb = const.tile([128, ct], f32)
        nc.scalar.copy(out=off_sb[:], in_=off8[:])
        w_sb = const.tile([128, ct], f32)
        nc.scalar.copy(out=w_sb[:], in_=w16[:])

        # build iota row [p,j]=j and identity [p,j]=(p==j) on device
        iota_i = const.tile([128, WIN], i32)
        nc.gpsimd.iota(iota_i[:], pattern=[[1, WIN]], base=0, channel_multiplier=0)
        iota_sb = const.tile([128, WIN], f32)
        nc.scalar.copy(out=iota_sb[:], in_=iota_i[:])
        part_i = const.tile([128, WIN], i32)
        nc.gpsimd.iota(part_i[:], pattern=[[0, WIN]], base=0, channel_multiplier=1)
        part_f = const.tile([128, WIN], f32)
        nc.scalar.copy(out=part_f[:], in_=part_i[:])
        ident_sb = const.tile([128, 128], f32)
        nc.vector.tensor_tensor(
            out=ident_sb[:], in0=iota_sb[:], in1=part_f[:], op=mybir.AluOpType.is_equal
        )

        partial = dram.tile([N_CORES * PAD, D], f32)
        hnb = dram.tile([PAD, D], f32)
        nf16t = dram.tile([PAD, D], f16)

        # widen the int8 shard to f16 in DRAM once (int values -127..127; the
        # per-row dequant scale is folded into edge weights / final act scale)
        for t in range(NW_OWN):
            ld8 = wk.tile([WIN, D], mybir.dt.int8, tag="ld8")
            nc.sync.dma_start(out=ld8[:], in_=nfeat_d[t * WIN : (t + 1) * WIN, :])
            cv = wk.tile([WIN, D], f16, tag="cv")
            nc.scalar.copy(out=cv[:], in_=ld8[:])
            nc.sync.dma_start(out=nf16t[t * WIN : (t + 1) * WIN, :], in_=cv[:])

        # Phase A: partial segment sums into every global dst window
        start = 0
        for t in range(NW):
            c = c_list[t]
            acc = ps.tile([WIN, D], f32, tag="acc")
            for j in range(c):
                col = start + j
                g = gp.tile([128, D], f16, tag="g")
                nc.gpsimd.indirect_dma_start(
                    out=g[:],
                    out_offset=None,
                    in_=nf16t[:],
                    in_offset=bass.IndirectOffsetOnAxis(
                        ap=src_sb[:, col : col + 1], axis=0
                    ),
                )
                a_t = ap.tile([128, WIN], f16, tag="A")
                nc.vector.tensor_scalar(
                    a_t[:],
                    iota_sb[:],
                    off_sb[:, col : col + 1],
                    w_sb[:, col : col + 1],
                    mybir.AluOpType.is_equal,
                    mybir.AluOpType.mult,
                )
                nc.tensor.matmul(
                    out=acc[:],
                    lhsT=a_t[:],
                    rhs=g[:],
                    start=(j == 0),
                    stop=(j == c - 1),
                )
            hb = wk.tile([WIN, D], f32, tag="hb")
            nc.scalar.copy(out=hb[:], in_=acc[:])
            nc.sync.dma_start(out=partial[t * WIN : (t + 1) * WIN, :], in_=hb[:])
            start += c

        # Phase B: sum partials across cores; each core keeps its own rows
        nc.gpsimd.collective_compute(
            "ReduceScatter",
            mybir.AluOpType.add,
            replica_groups=[list(range(N_CORES))],
            ins=[partial.opt()],
            outs=[hnb.opt()],
        )

        # Phase C: finalize owned windows; ship int8 outputs + per-row absmax
        rs_sb = const.tile([128, NW_OWN], f32)
        for t in range(NW_OWN):
            nf16 = wk.tile([WIN, D], f16, tag="nf16")
            nc.sync.dma_start(out=nf16[:], in_=nf16t[t * WIN : (t + 1) * WIN, :])
            hw = wk.tile([WIN, D], f32, tag="hw")
            nc.sync.dma_start(out=hw[:], in_=hnb[t * WIN : (t + 1) * WIN, :])
            nf = wk.tile([WIN, D], f32, tag="nf")
            nc.scalar.copy(out=nf[:], in_=nf16[:])
            x = wk.tile([WIN, D], f32, tag="x")
            nc.vector.tensor_tensor(
                out=x[:], in0=nf[:], in1=hw[:], op=mybir.AluOpType.mult
            )
            xt_ps = ps.tile([D, WIN], f32, tag="xt")
            nc.tensor.transpose(out=xt_ps[:], in_=x[:], identity=ident_sb[:])
            xt = wk.tile([D, WIN], f32, tag="xts")
            nc.scalar.copy(out=xt[:], in_=xt_ps[:])
            op_ps = ps.tile([WIN, D], f32, tag="op")
            nc.tensor.matmul(
                out=op_ps[:], lhsT=xt[:], rhs=wt_sb[:], start=True, stop=True
            )
            ob32 = wk.tile([WIN, D], f32, tag="ob32")
            # fold the per-row int8 dequant scale in here: for s>0,
            # lrelu(s*y) == s*lrelu(y), and row r of X@W^T scales by s_r
            nc.scalar.activation(
                out=ob32[:],
                in_=op_ps[:],
                func=mybir.ActivationFunctionType.Lrelu,
                scale=scale_sb[:, t : t + 1],
                alpha=0.01,
            )
            # int8 row-quantized wire format: rm = absmax(row), out = y*127/rm
            rm = wk.tile([WIN, 1], f32, tag="rm")
            nc.vector.tensor_reduce(
                out=rm[:], in_=ob32[:], axis=mybir.AxisListType.X,
                op=mybir.AluOpType.max, apply_absolute_value=True,
            )
            nc.scalar.copy(out=rs_sb[:, t : t + 1], in_=rm[:])
            rmg = wk.tile([WIN, 1], f32, tag="rmg")
            nc.vector.tensor_scalar(
                rmg[:], rm[:], 1e-30, None, mybir.AluOpType.add
            )
            inv = wk.tile([WIN, 1], f32, tag="inv")
            nc.vector.reciprocal(out=inv[:], in_=rmg[:])
            ob = wk.tile([WIN, D], mybir.dt.int8, tag="ob")
            nc.vector.tensor_scalar(
                ob[:], ob32[:], inv[:, 0:1], 127.0,
                mybir.AluOpType.mult, mybir.AluOpType.mult,
            )
            nc.sync.dma_start(out=out_d[t * WIN : (t + 1) * WIN, :], in_=ob[:])
        nc.sync.dma_start(out=outs_d[:], in_=rs_sb[:])
    _split_excess_waits(nc)
    return nc


def _kernel_impl(nfeat, edge_src, edge_dst, edge_w, W, npc=NPC, trace=False):
    n, d = nfeat.shape
    assert d == D and npc == NPC and npc * N_CORES == n
    E = edge_src.shape[0]

    src = np.asarray(edge_src, dtype=np.int32)
    dst = np.asarray(edge_dst, dtype=np.int32)
    w = np.asarray(edge_w, dtype=np.float32)
    nfeat = np.asarray(nfeat, dtype=np.float32)

    # per-row symmetric int8 quantization of nfeat; the dequant scale is
    # folded into edge weights (message path) and final act scale (X path).
    # Runs on a worker thread (numpy releases the GIL on these ufuncs) while
    # the main thread does the edge bucketing -- the chains are independent.
    quant = {}

    def _quantize():
        absmax = np.maximum(nfeat.max(axis=1), -nfeat.min(axis=1))
        scale = np.maximum(absmax, 1e-12) * (1.0 / 127.0)  # [n]
        q8f = nfeat * (1.0 / scale)[:, None]
        np.rint(q8f, out=q8f)
        nfeat_pad = np.zeros((N_CORES, PAD, D), np.int8)
        np.copyto(
            nfeat_pad[:, :NPC], q8f.reshape(N_CORES, NPC, D), casting="unsafe"
        )
        # scale [128, NW_OWN]: partition p, col t  <->  own row t*128+p
        scale_pad = np.zeros((N_CORES, PAD), np.float32)
        scale_pad[:, :NPC] = scale.reshape(N_CORES, NPC)
        quant["scale"] = scale
        quant["nfeat_pad"] = nfeat_pad
        quant["scale_arr"] = np.ascontiguousarray(
            scale_pad.reshape(N_CORES, NW_OWN, 128).transpose(0, 2, 1)
        )

    qfut = _pool.submit(_quantize)

    owner = src // NPC
    src_local = (src - owner * NPC).astype(np.int16)
    kd = dst // NPC
    prow = kd * PAD + (dst - kd * NPC)
    win = prow >> 7
    off = (prow & 127).astype(np.uint8)

    key = (owner * NW + win).astype(np.int16)  # values < 6272
    order = np.argsort(key, kind="stable")  # 16-bit radix sort, ~20ms
    ks = key[order].astype(np.int32)

    cnt = np.bincount(key, minlength=N_CORES * NW).reshape(N_CORES, NW)
    c_arr = np.maximum(1, -(-cnt // 128)).max(axis=0).astype(np.int32)  # [NW]
    c_list = [int(v) for v in c_arr]
    ct = int(c_arr.sum())
    col0 = np.concatenate([[0], np.cumsum(c_arr)[:-1]]).astype(np.int32)  # [NW]
    bstart = np.concatenate([[0], np.cumsum(cnt.ravel())])[:-1].astype(np.int32)
    rank = np.arange(E, dtype=np.int32) - bstart[ks]
    owner_s = ks // NW
    win_s = ks - owner_s * NW
    col = col0[win_s] + (rank >> 7)
    row = rank & 127
    flat = (owner_s * 128 + row) * ct + col

    src_arr = np.zeros(N_CORES * 128 * ct, np.int16)
    src_arr[flat] = src_local[order]
    src_arr = src_arr.reshape(N_CORES, 128, ct)
    off_arr = np.zeros(N_CORES * 128 * ct, np.uint8)
    off_arr[flat] = off[order]
    off_arr = off_arr.reshape(N_CORES, 128, ct)

    wt = np.ascontiguousarray(np.asarray(W).T.astype(np.float32))
    qfut.result()
    scale = quant["scale"]
    nfeat_pad = quant["nfeat_pad"]
    scale_arr = quant["scale_arr"]

    wp = (w * scale[src]).astype(np.float16)  # fold src-row dequant scale
    w_arr = np.zeros(N_CORES * 128 * ct, np.float16)
    w_arr[flat] = wp[order]
    w_arr = w_arr.reshape(N_CORES, 128, ct)

    key_nc = (ct, tuple(c_list))
    if key_nc not in _nc_cache:
        nc_new = _build_nc(ct, c_list)
        # the BIR is immutable after build; memoize its (re-)serialization,
        # which lowering otherwise redoes on every call (~0.13s)
        jb = nc_new.to_json_bytes()
        nc_new.to_json_bytes = lambda: jb
        _nc_cache[key_nc] = nc_new
    nc = _nc_cache[key_nc]

    in_maps = []
    for k in range(N_CORES):
        in_maps.append(
            {
                "nfeat": nfeat_pad[k],
                "scl": scale_arr[k],
                "src": src_arr[k],
                "offs": off_arr[k],
                "wf": w_arr[k],
                "wt": wt,
            }
        )

    r = run_bass_kernel_spmd(nc, in_maps, list(range(N_CORES)), trace=trace)
    out = np.empty((n, D), np.float32)

    def _assemble(k):
        o8 = r.results[k]["out"][:NPC]  # int8 [NPC, D]
        rs = np.asarray(r.results[k]["outs"])  # [128, NW_OWN] row absmax
        rowscale = (rs.T.reshape(PAD)[:NPC] * (1.0 / 127.0))[:, None]
        np.multiply(o8, rowscale, out=out[k * NPC : (k + 1) * NPC], casting="unsafe")

    # numpy ufuncs release the GIL on arrays this large; threads overlap
    list(_pool.map(_assemble, range(N_CORES)))
    if trace:
        return out, r
    return out


def kernel(nfeat, edge_src, edge_dst, edge_w, W):
    return _kernel_impl(
        np.asarray(nfeat),
        np.asarray(edge_src),
        np.asarray(edge_dst),
        np.asarray(edge_w),
        np.asarray(W),
        npc=NPC,
    )



# revision 3
# speedup vs baseline: 1.4164x; 1.4164x over previous
"""KGATConv GNN message-passing kernel for 8 Trainium2 NeuronCores.

Strategy (dst-node sharding + on-device AllGather; wire-optimized):
  - Core k owns dst nodes [k*12500, (k+1)*12500) and receives ONLY the edges
    whose dst it owns, so each core's segment-sum over its 98 dst windows is
    COMPLETE locally -- no cross-core reduction of partials.
  - Each core ships only its own nfeat shard (int8 row-quantized, 1.6MB); an
    on-device AllGather (fast NeuronLink, not the slow axon tunnel) builds the
    full [100352,128] int8 node table every core gathers src rows from.
  - Edge tables ship as 3 uint8 planes of a 24-bit pack (src_row<<7 | dst_off)
    plus an f16 weight (w * dequant_scale[src]); the device unpacks with
    shifts/ands.  Since edges bucket by dst window only (98 windows, ~2040
    edges each), chunk-of-128 padding waste is ~6% (vs ~50% for the 784-window
    src+dst bucketing this replaces).
  - Device, per own window t: per chunk, indirect-DMA gather of 128 int8 rows
    (widened to f16 on ACT); DVE builds A[p,j] = w_p * (dstoff_p==j); PE
    matmul-accumulates h_nb = A^T @ g in PSUM f32.  Finalize inline: X =
    nfeat_own * h_nb, X^T via PE transpose, out = X @ W^T, LeakyReLU on ACT
    (own-row dequant scale folded in), int8 row-quantized out on the wire.
  - Runner avoids run_bass_kernel_spmd's donated host zeros (13MB of zeros
    over the ~50MB/s axon tunnel per call): output-named operands are cached
    device-resident buffers, reused non-donated (every output element is
    written by the kernel, so their content is irrelevant).
  - nfeat H2D starts on a worker thread as soon as quantization finishes,
    overlapping the edge bucketing on the main thread; outputs are fetched
    per-shard in parallel and assembled threaded.
"""

import sys

sys.path.insert(0, "/opt/trn_rl_repo")

from concurrent.futures import ThreadPoolExecutor
from contextlib import ExitStack

import numpy as np
import jax
import jax.numpy as jnp

# Persistent compilation cache: without it every fresh process re-enters
# neuronx_cc_hook (walrus birverifier subprocess + DVE table gen).
for _k, _v in (
    ("jax_compilation_cache_dir", "/tmp/jax_pcc"),
    ("jax_persistent_cache_min_compile_time_secs", 0),
    ("jax_persistent_cache_min_entry_size_bytes", 0),
):
    try:
        jax.config.update(_k, _v)
    except Exception:
        pass

from jax.sharding import Mesh, NamedSharding, PartitionSpec

from jax.experimental.shard_map import shard_map

import concourse.bass as bass
import concourse.mybir as mybir
import concourse.tile as tile
import concourse.bass2jax as b2j

N_CORES = 8
D = 128
WIN = 128
NPC = 12500  # nodes owned per core
PAD = 12544  # NPC rounded up to a whole number of 128-row windows
NW_OWN = PAD // WIN  # 98 dst windows per core
NROWS = N_CORES * PAD  # rows in the AllGathered node table

_entry_cache = {}
_pool = ThreadPoolExecutor(max_workers=N_CORES)
_mesh_cache = {}


def _mesh():
    if "m" not in _mesh_cache:
        devices = jax.devices()[:N_CORES]
        mesh = Mesh(np.asarray(devices), ("core",))
        _mesh_cache["m"] = mesh
        _mesh_cache["sh"] = NamedSharding(mesh, PartitionSpec("core"))
    return _mesh_cache["m"], _mesh_cache["sh"]


def _split_excess_waits(nc, maxw=1):
    # This walrus build rejects instructions carrying more than one sync
    # wait.  Move extras onto the immediately preceding instruction of the
    # same engine+queue when it has a free wait slot (engine queues are
    # in-order, so hoisting a monotonic-semaphore wait one slot earlier is
    # equivalent to the NoOp the fallback inserts); otherwise insert NoOps.
    def qkey(i):
        return (i.engine, getattr(i, "queue", None))

    for f in nc.m.functions:
        for bb in f.blocks:
            out = []
            for inst in bb.instructions:
                si = inst.sync_info
                waits = list(si.on_wait) if si and si.on_wait else []
                if len(waits) > maxw:
                    extra, keep = waits[:-maxw], waits[-maxw:]
                    k = len(out) - 1
                    while extra and k >= 0 and qkey(out[k]) == qkey(inst):
                        psi = out[k].sync_info
                        pw = list(psi.on_wait) if psi and psi.on_wait else []
                        room = maxw - len(pw)
                        if room <= 0:
                            break
                        take, extra = extra[-room:], extra[:-room]
                        if psi is None:
                            out[k].sync_info = type(si)(
                                on_wait=list(take), on_update=[]
                            )
                        else:
                            psi.on_wait = pw + list(take)
                        k -= 1
                    for i in range(0, len(extra), maxw):
                        nop = mybir.InstNoOp(
                            name=nc.get_next_instruction_name(), ins=[], outs=[]
                        )
                        nop.engine = inst.engine
                        nop.sync_info = type(si)(
                            on_wait=extra[i : i + maxw], on_update=[]
                        )
                        nc.register_instruction(nop, overwrite=True)
                        out.append(nop)
                    si.on_wait = keep
                out.append(inst)
            bb.instructions[:] = out


def _build_nc(ct, c_list):
    f32 = mybir.dt.float32
    f16 = mybir.dt.float16
    i32 = mybir.dt.int32
    u8 = mybir.dt.uint8
    i8 = mybir.dt.int8
    nc = bass.Bass(num_devices=N_CORES)
    nfeat_d = nc.declare_dram_parameter("nfeat", [PAD, D], i8, isOutput=False)
    scale_d = nc.declare_dram_parameter("scl", [128, NW_OWN], f32, isOutput=False)
    p0_d = nc.declare_dram_parameter("p0", [128, ct], u8, isOutput=False)
    p1_d = nc.declare_dram_parameter("p1", [128, ct], u8, isOutput=False)
    p2_d = nc.declare_dram_parameter("p2", [128, ct], u8, isOutput=False)
    w_d = nc.declare_dram_parameter("wf", [128, ct], f16, isOutput=False)
    wt_d = nc.declare_dram_parameter("wt", [D, D], f32, isOutput=False)
    out_d = nc.declare_dram_parameter("out", [PAD, D], i8, isOutput=True)
    outs_d = nc.declare_dram_parameter("outs", [128, NW_OWN], f32, isOutput=True)

    with tile.TileContext(nc) as tc, ExitStack() as ctx:
        const = ctx.enter_context(tc.tile_pool(name="const", bufs=1))
        up = ctx.enter_context(tc.tile_pool(name="up", bufs=2))
        gp = ctx.enter_context(tc.tile_pool(name="gp", bufs=10))
        ap = ctx.enter_context(tc.tile_pool(name="ap", bufs=4))
        wk = ctx.enter_context(tc.tile_pool(name="wk", bufs=3))
        ps = ctx.enter_context(tc.tile_pool(name="ps", bufs=2, space="PSUM"))
        dram = ctx.enter_context(tc.tile_pool(name="dram", bufs=1, space="DRAM"))

        # ---- AllGather the int8 node table (collectives can't read IO
        # tensors, so bounce the shard through an internal DRAM tile) ----
        nfloc = dram.tile([PAD, D], i8)
        nc.sync.dma_start(out=nfloc[:], in_=nfeat_d[:])
        table = dram.tile([NROWS, D], i8)
        nc.gpsimd.collective_compute(
            "AllGather",
            mybir.AluOpType.bypass,
            replica_groups=[list(range(N_CORES))],
            ins=[nfloc[:].opt()],
            outs=[table[:].opt()],
        )

        # ---- load + unpack edge tables ----
        p0_sb = const.tile([128, ct], u8)
        nc.sync.dma_start(out=p0_sb[:], in_=p0_d[:])
        p1_sb = const.tile([128, ct], u8)
        nc.sync.dma_start(out=p1_sb[:], in_=p1_d[:])
        p2_sb = const.tile([128, ct], u8)
        nc.sync.dma_start(out=p2_sb[:], in_=p2_d[:])
        w16 = const.tile([128, ct], f16)
        nc.sync.dma_start(out=w16[:], in_=w_d[:])
        scale_sb = const.tile([128, NW_OWN], f32)
        nc.sync.dma_start(out=scale_sb[:], in_=scale_d[:])
        wt_sb = const.tile([D, D], f32)
        nc.sync.dma_start(out=wt_sb[:], in_=wt_d[:])

        # A24 = p0 + (p1<<8) + (p2<<16); src row = A24>>7; dst off = A24&127
        p0i = up.tile([128, ct], i32, tag="p0i")
        nc.scalar.copy(out=p0i[:], in_=p0_sb[:])
        p1i = up.tile([128, ct], i32, tag="p1i")
        nc.scalar.copy(out=p1i[:], in_=p1_sb[:])
        p2i = up.tile([128, ct], i32, tag="p2i")
        nc.scalar.copy(out=p2i[:], in_=p2_sb[:])
        nc.vector.tensor_scalar(
            p1i[:], p1i[:], 8, None, mybir.AluOpType.arith_shift_left
        )
        nc.vector.tensor_scalar(
            p2i[:], p2i[:], 16, None, mybir.AluOpType.arith_shift_left
        )
        a24 = up.tile([128, ct], i32, tag="a24")
        nc.vector.tensor_tensor(
            out=a24[:], in0=p0i[:], in1=p1i[:], op=mybir.AluOpType.add
        )
        nc.vector.tensor_tensor(
            out=a24[:], in0=a24[:], in1=p2i[:], op=mybir.AluOpType.add
        )
        src_sb = const.tile([128, ct], i32)
        nc.vector.tensor_scalar(
            src_sb[:], a24[:], 7, None, mybir.AluOpType.logical_shift_right
        )
        offi = up.tile([128, ct], i32, tag="offi")
        nc.vector.tensor_scalar(
            offi[:], a24[:], 127, None, mybir.AluOpType.bitwise_and
        )
        off_sb = const.tile([128, ct], f32)
        nc.scalar.copy(out=off_sb[:], in_=offi[:])
        w_sb = const.tile([128, ct], f32)
        nc.scalar.copy(out=w_sb[:], in_=w16[:])

        # iota row [p,j]=j and identity [p,j]=(p==j)
        iota_i = const.tile([128, WIN], i32)
        nc.gpsimd.iota(iota_i[:], pattern=[[1, WIN]], base=0, channel_multiplier=0)
        iota_sb = const.tile([128, WIN], f32)
        nc.scalar.copy(out=iota_sb[:], in_=iota_i[:])
        part_i = const.tile([128, WIN], i32)
        nc.gpsimd.iota(part_i[:], pattern=[[0, WIN]], base=0, channel_multiplier=1)
        part_f = const.tile([128, WIN], f32)
        nc.scalar.copy(out=part_f[:], in_=part_i[:])
        ident_sb = const.tile([128, 128], f32)
        nc.vector.tensor_tensor(
            out=ident_sb[:], in0=iota_sb[:], in1=part_f[:], op=mybir.AluOpType.is_equal
        )

        # ---- fused segment-sum + bi-interaction per own dst window ----
        rs_sb = const.tile([128, NW_OWN], f32)
        start = 0
        for t in range(NW_OWN):
            c = c_list[t]
            acc = ps.tile([WIN, D], f32, tag="acc")
            for j in range(c):
                col = start + j
                g8 = gp.tile([128, D], i8, tag="g8")
                nc.gpsimd.indirect_dma_start(
                    out=g8[:],
                    out_offset=None,
                    in_=table[:],
                    in_offset=bass.IndirectOffsetOnAxis(
                        ap=src_sb[:, col : col + 1], axis=0
                    ),
                )
                g16 = gp.tile([128, D], f16, tag="g16")
                nc.scalar.copy(out=g16[:], in_=g8[:])
                a_t = ap.tile([128, WIN], f16, tag="A")
                nc.vector.tensor_scalar(
                    a_t[:],
                    iota_sb[:],
                    off_sb[:, col : col + 1],
                    w_sb[:, col : col + 1],
                    mybir.AluOpType.is_equal,
                    mybir.AluOpType.mult,
                )
                nc.tensor.matmul(
                    out=acc[:],
                    lhsT=a_t[:],
                    rhs=g16[:],
                    start=(j == 0),
                    stop=(j == c - 1),
                )
            start += c

            nf8 = wk.tile([WIN, D], i8, tag="nf8")
            nc.sync.dma_start(out=nf8[:], in_=nfeat_d[t * WIN : (t + 1) * WIN, :])
            nf = wk.tile([WIN, D], f32, tag="nf")
            nc.scalar.copy(out=nf[:], in_=nf8[:])
            hb = wk.tile([WIN, D], f32, tag="hb")
            nc.scalar.copy(out=hb[:], in_=acc[:])
            x = wk.tile([WIN, D], f32, tag="x")
            nc.vector.tensor_tensor(
                out=x[:], in0=nf[:], in1=hb[:], op=mybir.AluOpType.mult
            )
            xt_ps = ps.tile([D, WIN], f32, tag="xt")
            nc.tensor.transpose(out=xt_ps[:], in_=x[:], identity=ident_sb[:])
            xt = wk.tile([D, WIN], f32, tag="xts")
            nc.scalar.copy(out=xt[:], in_=xt_ps[:])
            op_ps = ps.tile([WIN, D], f32, tag="op")
            nc.tensor.matmul(
                out=op_ps[:], lhsT=xt[:], rhs=wt_sb[:], start=True, stop=True
            )
            ob32 = wk.tile([WIN, D], f32, tag="ob32")
            # fold the per-row int8 dequant scale in here: for s>0,
            # lrelu(s*y) == s*lrelu(y), and row r of X@W^T scales by s_r
            nc.scalar.activation(
                out=ob32[:],
                in_=op_ps[:],
                func=mybir.ActivationFunctionType.Lrelu,
                scale=scale_sb[:, t : t + 1],
                alpha=0.01,
            )
            # int8 row-quantized wire format: rm = absmax(row), out = y*127/rm
            rm = wk.tile([WIN, 1], f32, tag="rm")
            nc.vector.tensor_reduce(
                out=rm[:], in_=ob32[:], axis=mybir.AxisListType.X,
                op=mybir.AluOpType.max, apply_absolute_value=True,
            )
            nc.scalar.copy(out=rs_sb[:, t : t + 1], in_=rm[:])
            rmg = wk.tile([WIN, 1], f32, tag="rmg")
            nc.vector.tensor_scalar(
                rmg[:], rm[:], 1e-30, None, mybir.AluOpType.add
            )
            inv = wk.tile([WIN, 1], f32, tag="inv")
            nc.vector.reciprocal(out=inv[:], in_=rmg[:])
            ob = wk.tile([WIN, D], i8, tag="ob")
            nc.vector.tensor_scalar(
                ob[:], ob32[:], inv[:, 0:1], 127.0,
                mybir.AluOpType.mult, mybir.AluOpType.mult,
            )
            nc.sync.dma_start(out=out_d[t * WIN : (t + 1) * WIN, :], in_=ob[:])
        nc.sync.dma_start(out=outs_d[:], in_=rs_sb[:])
    _split_excess_waits(nc)
    return nc


def _get_entry(ct, c_list):
    key = (ct, tuple(c_list))
    if key in _entry_cache:
        return _entry_cache[key]

    nc = _build_nc(ct, c_list)
    jb = nc.to_json_bytes()
    nc.to_json_bytes = lambda: jb

    b2j.install_neuronx_cc_hook()
    partition_name = nc.partition_id_tensor.name if nc.partition_id_tensor else None
    in_names, out_names, out_avals = [], [], []
    for alloc in nc.m.functions[0].allocations:
        if not isinstance(alloc, mybir.MemoryLocationSet):
            continue
        name = alloc.memorylocations[0].name
        if alloc.kind == "ExternalInput":
            if name != partition_name:
                in_names.append(name)
        elif alloc.kind == "ExternalOutput":
            out_names.append(name)
            out_avals.append(
                jax.core.ShapedArray(
                    tuple(alloc.tensor_shape), mybir.dt.np(alloc.dtype)
                )
            )
    n_params = len(in_names)
    all_names = list(in_names) + out_names
    if partition_name is not None:
        all_names.append(partition_name)

    def _body(*args):
        operands = list(args)
        if partition_name is not None:
            operands.append(b2j.partition_id_tensor())
        return tuple(
            b2j._bass_exec_p.bind(
                *operands,
                out_avals=tuple(out_avals),
                in_names=tuple(all_names),
                out_names=tuple(out_names),
                lowering_input_output_aliases=(),
                sim_require_finite=True,
                sim_require_nnan=True,
                nc=nc,
            )
        )

    mesh, sh = _mesh()
    n_outs = len(out_avals)
    sharded = jax.jit(
        shard_map(
            _body,
            mesh=mesh,
            in_specs=(PartitionSpec("core"),) * (n_params + n_outs),
            out_specs=(PartitionSpec("core"),) * n_outs,
            check_rep=False,
        )
    )
    # Output-named operands: the NEFF writes every element of both outputs,
    # so these buffers only need the right shape.  Create them ON DEVICE once
    # and reuse non-donated -- never ship 13MB of zeros over the tunnel.
    zmake = jax.jit(
        lambda: tuple(
            jnp.zeros((N_CORES * a.shape[0], *a.shape[1:]), a.dtype)
            for a in out_avals
        ),
        out_shardings=tuple(sh for _ in out_avals),
    )
    zbufs = zmake()
    entry = {
        "sharded": sharded,
        "in_names": in_names,
        "out_names": out_names,
        "zbufs": zbufs,
    }
    _entry_cache[key] = entry
    return entry


def _kernel_impl(nfeat, edge_src, edge_dst, edge_w, W, npc=NPC, trace=False):
    n, d = nfeat.shape
    assert d == D and npc == NPC and npc * N_CORES == n
    E = edge_src.shape[0]

    src = np.asarray(edge_src, dtype=np.int32)
    dst = np.asarray(edge_dst, dtype=np.int32)
    w = np.asarray(edge_w, dtype=np.float32)
    nfeat = np.asarray(nfeat, dtype=np.float32)

    # per-row symmetric int8 quantization of nfeat; the dequant scale is
    # folded into edge weights (message path) and final act scale (X path).
    quant = {}

    def _quantize():
        absmax = np.maximum(nfeat.max(axis=1), -nfeat.min(axis=1))
        scale = np.maximum(absmax, 1e-12) * (1.0 / 127.0)  # [n]
        q8f = nfeat * (1.0 / scale)[:, None]
        np.rint(q8f, out=q8f)
        nfeat_pad = np.zeros((N_CORES, PAD, D), np.int8)
        np.copyto(
            nfeat_pad[:, :NPC], q8f.reshape(N_CORES, NPC, D), casting="unsafe"
        )
        scale_pad = np.zeros((N_CORES, PAD), np.float32)
        scale_pad[:, :NPC] = scale.reshape(N_CORES, NPC)
        quant["scale"] = scale
        quant["nfeat_pad"] = nfeat_pad.reshape(N_CORES * PAD, D)
        # scale [128, NW_OWN] per core: partition p, col t <-> own row t*128+p
        quant["scale_arr"] = np.ascontiguousarray(
            scale_pad.reshape(N_CORES, NW_OWN, 128).transpose(0, 2, 1)
        ).reshape(N_CORES * 128, NW_OWN)

    qfut = _pool.submit(_quantize)

    # start the 12.8MB nfeat H2D as early as possible, off the main thread
    def _put_nfeat():
        qfut.result()
        _, sh = _mesh()
        return jax.device_put(quant["nfeat_pad"], sh)

    put_fut = _pool.submit(_put_nfeat)

    # ---- bucket edges by dst owner + own dst window ----
    kd = dst // NPC
    r = dst - kd * NPC
    t = r >> 7
    off = r & 127
    key = (kd * NW_OWN + t).astype(np.int16)  # values < 784
    order = np.argsort(key, kind="stable")  # 16-bit radix sort
    ks = key[order].astype(np.int32)

    cnt = np.bincount(key, minlength=N_CORES * NW_OWN).reshape(N_CORES, NW_OWN)
    c_arr = np.maximum(1, -(-cnt // 128)).max(axis=0).astype(np.int32)  # [98]
    c_list = [int(v) for v in c_arr]
    ct = int(c_arr.sum())
    col0 = np.concatenate([[0], np.cumsum(c_arr)[:-1]]).astype(np.int32)
    bstart = np.concatenate([[0], np.cumsum(cnt.ravel())])[:-1].astype(np.int32)
    rank = np.arange(E, dtype=np.int32) - bstart[ks]
    k_s = ks // NW_OWN
    t_s = ks - k_s * NW_OWN
    col = col0[t_s] + (rank >> 7)
    row = rank & 127
    flat = (k_s * 128 + row) * ct + col

    # 24-bit pack: (global padded src row) << 7 | dst offset in window
    so = src // NPC
    grow = so * PAD + (src - so * NPC)
    a24 = (grow << 7) | off
    a24_s = a24[order]
    p0_arr = np.zeros(N_CORES * 128 * ct, np.uint8)
    p0_arr[flat] = (a24_s & 255).astype(np.uint8)
    p1_arr = np.zeros(N_CORES * 128 * ct, np.uint8)
    p1_arr[flat] = ((a24_s >> 8) & 255).astype(np.uint8)
    p2_arr = np.zeros(N_CORES * 128 * ct, np.uint8)
    p2_arr[flat] = (a24_s >> 16).astype(np.uint8)

    wt = np.ascontiguousarray(np.asarray(W).T.astype(np.float32))
    wt_g = np.broadcast_to(wt, (N_CORES, D, D)).reshape(N_CORES * D, D)
    wt_g = np.ascontiguousarray(wt_g)
    qfut.result()
    scale = quant["scale"]
    scale_arr = quant["scale_arr"]

    wp = (w * scale[src]).astype(np.float16)  # fold src-row dequant scale
    w_arr = np.zeros(N_CORES * 128 * ct, np.float16)
    w_arr[flat] = wp[order]

    entry = _get_entry(ct, c_list)
    arrays = {
        "nfeat": put_fut.result(),
        "scl": scale_arr,
        "p0": p0_arr.reshape(N_CORES * 128, ct),
        "p1": p1_arr.reshape(N_CORES * 128, ct),
        "p2": p2_arr.reshape(N_CORES * 128, ct),
        "wf": w_arr.reshape(N_CORES * 128, ct),
        "wt": wt_g,
    }
    args = [arrays[name] for name in entry["in_names"]]
    out_arrs = entry["sharded"](*args, *entry["zbufs"])
    o_by_name = dict(zip(entry["out_names"], out_arrs))

    out = np.empty((n, D), np.float32)
    o8_g = o_by_name["out"]
    rs_g = o_by_name["outs"]

    def _assemble(k):
        o8 = np.asarray(o8_g.addressable_shards[k].data)[:NPC]  # int8 [NPC,D]
        rs = np.asarray(rs_g.addressable_shards[k].data)  # [128, NW_OWN]
        kk = o8_g.addressable_shards[k].index[0].start // PAD
        rowscale = (rs.T.reshape(PAD)[:NPC] * (1.0 / 127.0))[:, None]
        np.multiply(o8, rowscale, out=out[kk * NPC : (kk + 1) * NPC], casting="unsafe")

    list(_pool.map(_assemble, range(N_CORES)))
    return out


def kernel(nfeat, edge_src, edge_dst, edge_w, W):
    return _kernel_impl(
        np.asarray(nfeat),
        np.asarray(edge_src),
        np.asarray(edge_dst),
        np.asarray(edge_w),
        np.asarray(W),
        npc=NPC,
    )
